# revision 24
# baseline (speedup 1.0000x reference)
"""ArcFace (non-linear squashing) + cross-entropy loss, distributed over 8 TRN2 NeuronCores.

Strategy (classic model-parallel ArcFace head):
  - Host folds the per-row squashing scale into x:  xs = x * sqrt(||x||^2)/(||x||^2+1)
    and the per-class L2 normalization into w:      wn = w / ||w||_row
    so that cosine = xs @ wn.T  with |cosine| <= 1 (no logsumexp max-shift needed:
    exp(30*cos) <= e^30 fits fp32 comfortably).
  - Classes (50000) are sharded column-wise across 8 cores (6250 each). The small
    xs is replicated. Both are quantized fp8-e4m3 and pre-transposed/interleaved so
    the contraction dim K=512 lands on SBUF partitions ([128, kc, *]: k = kc*128+p).
  - Each core: PE computes cosine tiles (fp8 DoubleRow, fp32 PSUM), ScalarE does
    exp(30*cos) -> bf16 with a free per-partition running sum (accum_out), VectorE
    tree-maxes the bf16 exp tile. The trailing 362 classes (256 rebalanced
    off the bottleneck ScalarE + the 106-class tail) bypass ScalarE: a
    single DVE tensor_scalar builds Schraudolph fast-exp bf16 bit patterns
    (pattern = round(A*cos + B) as int16), then two small reduces give their
    sum and max. Only [2, 128, 32] f32 leaves each core.
  - Host combines the 8 cores' per-(b_chunk, group) partial sums/maxes, applies
    the one-hot phi swap correction for the label column analytically, and forms
    (loss, acc). argmax(phi) == argmax(cosine) since phi is strictly increasing
    in cosine, so accuracy reduces to "is the label's cosine the row max".
"""

import math
import sys

import numpy as np

if "/opt/trn_rl_repo" not in sys.path:  # harmless if site config already provides it
    sys.path.insert(0, "/opt/trn_rl_repo")

import ml_dtypes

import concourse.bacc as bacc
import concourse.bass as bass
import concourse.mybir as mybir
from concourse import tile
from concourse.bass_utils import run_bass_kernel_spmd

# Problem constants (hardcoded per the harness contract)
B = 1024
K = 512
C = 50000
NCORES = 8
CSH = C // NCORES  # 6250 classes per core

M_MARGIN = 0.5
S = 30.0
COS_M = math.cos(M_MARGIN)
SIN_M = math.sin(M_MARGIN)
TH = math.cos(math.pi - M_MARGIN)
MM = math.sin(math.pi - M_MARGIN) * M_MARGIN

CG = 2048            # classes per ACT group (4 PSUM banks; 2 groups double-buffer)
NG = 3               # ACT groups per b-chunk (3*2048 = 6144)
TAIL = CSH - NG * CG  # 106 trailing classes
DVE_COLS = 256       # classes per b-chunk offloaded from ScalarE to the DVE
                     # fast-exp path (joined with TAIL into one 362-col region)
NCOL = 4             # stat columns per b-chunk: 3 ACT groups + 1 fast-exp region

# Schraudolph fast-exp in bf16-pattern domain: bf16bits(exp(S*c)) ~ FE_A*c + FE_B
FE_A = S / math.log(2.0) * 128.0
FE_C = 0.0573  # mean-centering bias (zeroes the average multiplicative error)
FE_B = (127.0 - FE_C) * 128.0

EX_BUFS = 3          # bf16 exp tile depth (hides the DVE tree under ACT/PE)
W_BUFS = 3           # weight tile prefetch depth

_NC_CACHE = {}


def build_nc(repeat=1, skip=(), cg=None, mm_n=None, sum_mode="accum", exbufs=None, wbufs=None):
    """Build + compile the per-core Bass program (same graph on all 8 cores).

    repeat > 1 re-runs the whole body N times inside one NEFF (benchmarking
    only - lets slope timing cancel per-execution dispatch overhead).
    skip: ("act",) and/or ("dve",) replace that engine's work with a light
    PSUM consumer - used to isolate per-engine rates when timing.
    cg: ACT group width (1024 or 2048); mm_n: matmul moving width;
    sum_mode: "accum" (default) sums each group via ScalarE accum_out;
    "none" drops sums (timing experiments only - sums garbage).
    (A fused DVE tensor_tensor_reduce sum was tried instead of accum_out,
    but InstTensorTensorReduce crashes this NEFF/runtime pipeline.)
    """
    cg = cg or CG
    mm_n = mm_n or 512
    dve_cols = DVE_COLS
    # per-b class layout: ng ACT groups, last one narrowed by dve_cols, then
    # a DVE fast-exp region of dve_cols + TAIL
    ng = (NG * CG) // cg
    widths = [cg] * ng
    widths[-1] -= dve_cols
    fx = dve_cols + TAIL       # fast-exp region width
    fx0 = CSH - fx             # its first class
    bf16 = mybir.dt.bfloat16
    f32 = mybir.dt.float32
    i16 = mybir.dt.int16
    in_dt = mybir.dt.float8e4

    nc = bacc.Bacc(
        "TRN2",
        target_bir_lowering=False,
        debug=False,
        num_devices=NCORES,
    )

    ncol = ng + 1
    xsT_d = nc.dram_tensor("xsT", [K, B], in_dt, kind="ExternalInput")
    wnT_d = nc.dram_tensor("wnT", [K, CSH], in_dt, kind="ExternalInput")
    out_d = nc.dram_tensor("out", [2, 128, 8 * ncol], f32, kind="ExternalOutput")

    with tile.TileContext(nc) as tc:
        with (
            tc.tile_pool(name="xs", bufs=1) as xs_pool,
            tc.tile_pool(name="w", bufs=wbufs or W_BUFS) as w_pool,
            tc.tile_pool(name="wt", bufs=1) as wt_pool,
            tc.tile_pool(name="ps", bufs=4096 // cg, space=bass.MemorySpace.PSUM) as ps_pool,
            tc.tile_pool(name="ex", bufs=exbufs or EX_BUFS) as ex_pool,
            tc.tile_pool(name="m1", bufs=2) as m1_pool,
            tc.tile_pool(name="m2", bufs=2) as m2_pool,
            tc.tile_pool(name="tx", bufs=2) as tx_pool,
            tc.tile_pool(name="st", bufs=1) as st_pool,
        ):
            # xs resident in SBUF as [p, kc, b]: k = kc*128 + p; two half-batch
            # DMAs so the first matmuls start after half the transfer.
            xs_sb = xs_pool.tile([128, 4, B], in_dt, tag="xs")
            xsT_r = xsT_d.ap().rearrange("(kc p) b -> p kc b", p=128)
            for half in range(2):
                sl = slice(half * (B // 2), (half + 1) * (B // 2))
                nc.sync.dma_start(xs_sb[:, :, sl], xsT_r[:, :, sl])

            # source view of wnT with partition inside: [p, kc, c]
            wnT_r = wnT_d.ap().rearrange("(kc p) c -> p kc c", p=128)

            # fast-exp region weights loaded once
            wt_sb = wt_pool.tile([128, 4, fx], in_dt, tag="wt")
            nc.sync.dma_start(wt_sb[:], wnT_r[:, :, fx0:CSH])

            # per-(b_chunk, group) partial stats; column b*ncol + gi
            sumbuf = st_pool.tile([128, 8 * ncol], f32, tag="sumbuf")
            maxbuf = st_pool.tile([128, 8 * ncol], f32, tag="maxbuf")
            nc.gpsimd.memset(maxbuf[:], 0.0)
            if skip or sum_mode == "none":
                nc.gpsimd.memset(sumbuf[:], 0.0)

            for _rep in range(repeat):
                for gi in range(ng):
                    c0 = gi * cg
                    w = widths[gi]
                    w_t = w_pool.tile([128, 4, cg], in_dt, tag="w")
                    # chunked so the first matmuls (sub-tile range deps) start
                    # as soon as their slice lands, shortening the ramp
                    nw = 4 if gi == 0 else 2
                    cw = (w + nw - 1) // nw
                    for ci in range(nw):
                        lo, hi = ci * cw, min((ci + 1) * cw, w)
                        if lo >= hi:
                            continue
                        nc.sync.dma_start(
                            w_t[:, :, lo:hi], wnT_r[:, :, c0 + lo : c0 + hi]
                        )
                    for b in range(8):
                        ps = ps_pool.tile([128, cg], f32, tag="ps")
                        # g outer / h inner: consecutive matmuls share the
                        # stationary operand (same b, g), easing LDWEIGHTS
                        # pressure; PSUM has_written bits handle the
                        # interleaved accumulation groups per h-slice.
                        for g in range(2):
                            for h0 in range(0, w, mm_n):
                                hsz = min(mm_n, w - h0)
                                nc.tensor.matmul(
                                    ps[:, h0 : h0 + hsz],
                                    xs_sb[:, 2 * g : 2 * g + 2, b * 128 : b * 128 + 128],
                                    w_t[:, 2 * g : 2 * g + 2, h0 : h0 + hsz],
                                    start=(g == 0),
                                    stop=(g == 1),
                                    perf_mode=mybir.MatmulPerfMode.DoubleRow,
                                    skip_group_check=True,
                                )
                        scol = sumbuf[:, b * ncol + gi : b * ncol + gi + 1]
                        if "act" in skip:
                            ex = ex_pool.tile([128, cg], bf16, tag="ex")
                            nc.scalar.activation(
                                ex[:, :8], ps[:, :8],
                                mybir.ActivationFunctionType.Copy, scale=1.0,
                            )
                        else:
                            ex = ex_pool.tile([128, cg], bf16, tag="ex")
                            nc.scalar.activation(
                                ex[:, :w],
                                ps[:, :w],
                                mybir.ActivationFunctionType.Exp,
                                scale=S,
                                accum_out=scol if sum_mode == "accum" else None,
                            )
                        mcol = maxbuf[:, b * ncol + gi : b * ncol + gi + 1]
                        if "dve" not in skip and "act" not in skip:
                            # free-axis max of the bf16 exp tile (exp monotone
                            # in cosine): shrink 8x with three 2x-packed
                            # tensor_tensor maxes, then a 1x-rate reduce.
                            # (DMA-ing partial-max slabs to the host instead
                            # costs ~3MB extra DMA and measured +15us.)
                            h1, h2, h3 = w // 2, w // 4, w // 8
                            t1 = m1_pool.tile([128, cg // 2], bf16, tag="mx1")
                            nc.vector.tensor_max(t1[:, :h1], ex[:, :h1], ex[:, h1:w])
                            t2 = m2_pool.tile([128, cg // 4], bf16, tag="mx2")
                            nc.vector.tensor_max(t2[:, :h2], t1[:, :h2], t1[:, h2:h1])
                            t3 = m2_pool.tile([128, cg // 8], bf16, tag="mx3")
                            nc.vector.tensor_max(t3[:, :h3], t2[:, :h3], t2[:, h3:h2])
                            nc.vector.tensor_reduce(
                                mcol, t3[:, :h3], axis=mybir.AxisListType.X,
                                op=mybir.AluOpType.max,
                            )

                # fast-exp region: dve_cols + TAIL classes via a DVE
                # Schraudolph fast-exp (ScalarE is the bottleneck engine;
                # this rebalances work onto VectorE's freed capacity)
                for b in range(8):
                    ps = ps_pool.tile([128, cg], f32, tag="ps")
                    for g in range(2):
                        for h0 in range(0, fx, mm_n):
                            hsz = min(mm_n, fx - h0)
                            nc.tensor.matmul(
                                ps[:, h0 : h0 + hsz],
                                xs_sb[:, 2 * g : 2 * g + 2, b * 128 : b * 128 + 128],
                                wt_sb[:, 2 * g : 2 * g + 2, h0 : h0 + hsz],
                                start=(g == 0),
                                stop=(g == 1),
                                perf_mode=mybir.MatmulPerfMode.DoubleRow,
                                skip_group_check=True,
                            )
                    scol = sumbuf[:, b * ncol + ng : b * ncol + ng + 1]
                    mcol = maxbuf[:, b * ncol + ng : b * ncol + ng + 1]
                    if "dve" in skip:
                        ex = ex_pool.tile([128, cg], bf16, tag="ex")
                        nc.scalar.activation(
                            ex[:, :8], ps[:, :8],
                            mybir.ActivationFunctionType.Copy, scale=1.0,
                        )
                    else:
                        tx = tx_pool.tile([128, fx], i16, tag="tx")
                        nc.vector.tensor_scalar(
                            tx[:], ps[:, :fx], FE_A, FE_B,
                            mybir.AluOpType.mult, mybir.AluOpType.add,
                        )
                        nc.vector.tensor_reduce(
                            scol, tx[:].bitcast(bf16), axis=mybir.AxisListType.X,
                            op=mybir.AluOpType.add,
                        )
                        nc.vector.tensor_reduce(
                            mcol, tx[:], axis=mybir.AxisListType.X,
                            op=mybir.AluOpType.max,
                        )

            out_ap = out_d.ap()
            nc.sync.dma_start(out_ap[0], sumbuf[:])
            nc.sync.dma_start(out_ap[1], maxbuf[:])

    nc.compile()
    return nc


def get_nc(repeat=1, skip=(), cg=None, mm_n=None, sum_mode="accum", exbufs=None, wbufs=None):
    key = (repeat, tuple(skip), cg or CG, mm_n or 512, sum_mode,
           exbufs or EX_BUFS, wbufs or W_BUFS)
    if key not in _NC_CACHE:
        _NC_CACHE[key] = build_nc(repeat, skip, cg, mm_n, sum_mode, exbufs, wbufs)
    return _NC_CACHE[key]


def quantize_host(x, w):
    """Host prep: fold squashing scale into x, L2 norm into w; quantize fp8;
    lay out as [K, B] / [K, C] with K rows (k = kc*128 + p after rearrange)."""
    qdt = ml_dtypes.float8_e4m3
    sq = np.einsum("bk,bk->b", x, x)
    xs = x * (np.sqrt(sq) / (sq + 1.0))[:, None]
    wn = w / np.sqrt(np.einsum("ck,ck->c", w, w))[:, None]
    xs_q = xs.astype(qdt)
    wn_q = wn.astype(qdt)
    xsT = np.ascontiguousarray(xs_q.T)  # [K, B]
    wnT = np.ascontiguousarray(wn_q.T)  # [K, C]
    return xs_q, wn_q, xsT, wnT


def make_in_maps(input, label, weight):
    x = np.asarray(input, dtype=np.float64)
    w = np.asarray(weight, dtype=np.float64)
    _, _, xsT, wnT = quantize_host(x, w)
    return [
        {"xsT": xsT, "wnT": np.ascontiguousarray(wnT[:, i * CSH : (i + 1) * CSH])}
        for i in range(NCORES)
    ]


def kernel(input, label, weight):
    x = np.asarray(input, dtype=np.float64)  # [B, K]
    lab = np.asarray(label).astype(np.int64)  # [B]
    w = np.asarray(weight, dtype=np.float64)  # [C, K]

    xs_q, wn_q, xsT, wnT = quantize_host(x, w)

    in_maps = [
        {"xsT": xsT, "wnT": np.ascontiguousarray(wnT[:, i * CSH : (i + 1) * CSH])}
        for i in range(NCORES)
    ]

    nc = get_nc()
    results = run_bass_kernel_spmd(nc, in_maps, core_ids=list(range(NCORES))).results

    # combine per-core partials: out[s][p, b*NCOL + g] is the stat for batch row
    # b*128 + p, class-group g (g<NG: ScalarE true-exp domain; g==NG: the
    # fast-exp region in Schraudolph bf16-pattern / fast-exp value domain)
    SE = np.zeros(B, dtype=np.float64)
    MXE = np.full(B, 0.0)    # max of exp(S*cos) over ACT groups (bf16-rounded)
    MXP = np.full(B, 0.0)    # max of fast-exp bit pattern over the DVE region
    for r in results:
        o = np.asarray(r["out"], dtype=np.float64)  # [2, 128, 8*NCOL]
        sums = o[0].T.reshape(8, NCOL, 128)   # [b_chunk, col, p]
        maxs = o[1].T.reshape(8, NCOL, 128)
        SE += sums.sum(axis=1).reshape(B)
        MXE = np.maximum(MXE, maxs[:, :NG].max(axis=1).reshape(B))
        MXP = np.maximum(MXP, maxs[:, NG].reshape(B))

    # row max cosine from the two max domains
    cos_act = np.log(np.maximum(MXE, 1e-300)) / S
    cos_tail = (MXP - FE_B) / FE_A
    cos_max = np.maximum(cos_act, cos_tail)

    # label-column correction on host, with the same quantized values the device saw
    xs_f = xs_q.astype(np.float64)
    wn_f = wn_q.astype(np.float64)
    coslab = np.einsum("bk,bk->b", xs_f, wn_f[lab])
    sine = np.sqrt(np.clip(1.0 - coslab * coslab, 0.0, 1.0))
    phi = np.where(coslab > TH, coslab * COS_M - sine * SIN_M, coslab - MM)

    total = SE - np.exp(S * coslab) + np.exp(S * phi)
    loss = np.mean(np.log(total) - S * phi)
    # acc: is the label's cosine the row max (slack covers bf16 rounding of the
    # exp domain, fast-exp pattern quantization, and fp22 matmul accumulation)
    acc = 100.0 * np.mean(coslab >= cos_max - 3e-4)

    return (np.float32(loss), np.float32(acc))


# revision 25
# speedup vs baseline: 1.0238x; 1.0238x over previous
"""ArcFace (non-linear squashing) + cross-entropy loss, distributed over 8 TRN2 NeuronCores.

Strategy (classic model-parallel ArcFace head):
  - Host folds the per-row squashing scale into x:  xs = x * sqrt(||x||^2)/(||x||^2+1)
    and the per-class L2 normalization into w:      wn = w / ||w||_row
    so that cosine = xs @ wn.T  with |cosine| <= 1 (no logsumexp max-shift needed:
    exp(30*cos) <= e^30 fits fp32 comfortably).
  - Classes (50000) are sharded column-wise across 8 cores (6250 each). The small
    xs is replicated. Both are quantized fp8-e4m3 and pre-transposed/interleaved so
    the contraction dim K=512 lands on SBUF partitions ([128, kc, *]: k = kc*128+p).
  - Each core: PE computes cosine tiles (fp8 DoubleRow, fp32 PSUM), ScalarE does
    exp(30*cos) -> bf16 with a free per-partition running sum (accum_out), VectorE
    tree-maxes the bf16 exp tile. The trailing 362 classes (256 rebalanced
    off the bottleneck ScalarE + the 106-class tail) bypass ScalarE: a
    single DVE tensor_scalar builds Schraudolph fast-exp bf16 bit patterns
    (pattern = round(A*cos + B) as int16), then two small reduces give their
    sum and max. Only [2, 128, 32] f32 leaves each core.
  - Host combines the 8 cores' per-(b_chunk, group) partial sums/maxes, applies
    the one-hot phi swap correction for the label column analytically, and forms
    (loss, acc). argmax(phi) == argmax(cosine) since phi is strictly increasing
    in cosine, so accuracy reduces to "is the label's cosine the row max".
"""

import math
import sys

import numpy as np

if "/opt/trn_rl_repo" not in sys.path:  # harmless if site config already provides it
    sys.path.insert(0, "/opt/trn_rl_repo")

import ml_dtypes

import concourse.bacc as bacc
import concourse.bass as bass
import concourse.mybir as mybir
from concourse import tile
from concourse.bass_utils import run_bass_kernel_spmd

# Problem constants (hardcoded per the harness contract)
B = 1024
K = 512
C = 50000
NCORES = 8
CSH = C // NCORES  # 6250 classes per core

M_MARGIN = 0.5
S = 30.0
COS_M = math.cos(M_MARGIN)
SIN_M = math.sin(M_MARGIN)
TH = math.cos(math.pi - M_MARGIN)
MM = math.sin(math.pi - M_MARGIN) * M_MARGIN

CG = 2048            # classes per ACT group (4 PSUM banks; 2 groups double-buffer)
NG = 3               # ACT groups per b-chunk (3*2048 = 6144)
TAIL = CSH - NG * CG  # 106 trailing classes
DVE_COLS = 256       # classes per b-chunk offloaded from ScalarE to the DVE
                     # fast-exp path (joined with TAIL into one 362-col region)
NCOL = 4             # stat columns per b-chunk: 3 ACT groups + 1 fast-exp region

# Schraudolph fast-exp in bf16-pattern domain: bf16bits(exp(S*c)) ~ FE_A*c + FE_B
FE_A = S / math.log(2.0) * 128.0
FE_C = 0.0573  # mean-centering bias (zeroes the average multiplicative error)
FE_B = (127.0 - FE_C) * 128.0

EX_BUFS = 3          # bf16 exp tile depth (hides the DVE tree under ACT/PE)
W_BUFS = 3           # weight tile prefetch depth

_NC_CACHE = {}


def build_nc(repeat=1, skip=(), cg=None, mm_n=None, sum_mode="accum", exbufs=None, wbufs=None):
    """Build + compile the per-core Bass program (same graph on all 8 cores).

    repeat > 1 re-runs the whole body N times inside one NEFF (benchmarking
    only - lets slope timing cancel per-execution dispatch overhead).
    skip: ("act",) and/or ("dve",) replace that engine's work with a light
    PSUM consumer - used to isolate per-engine rates when timing.
    cg: ACT group width (1024 or 2048); mm_n: matmul moving width;
    sum_mode: "accum" (default) sums each group via ScalarE accum_out;
    "none" drops sums (timing experiments only - sums garbage).
    (A fused DVE tensor_tensor_reduce sum was tried instead of accum_out,
    but InstTensorTensorReduce crashes this NEFF/runtime pipeline.)
    """
    cg = cg or CG
    mm_n = mm_n or 512
    dve_cols = DVE_COLS
    # per-b class layout: ng ACT groups, last one narrowed by dve_cols, then
    # a DVE fast-exp region of dve_cols + TAIL
    ng = (NG * CG) // cg
    widths = [cg] * ng
    widths[-1] -= dve_cols
    fx = dve_cols + TAIL       # fast-exp region width
    fx0 = CSH - fx             # its first class
    bf16 = mybir.dt.bfloat16
    f32 = mybir.dt.float32
    i16 = mybir.dt.int16
    in_dt = mybir.dt.float8e4

    nc = bacc.Bacc(
        "TRN2",
        target_bir_lowering=False,
        debug=False,
        num_devices=NCORES,
    )

    ncol = ng + 1
    xsT_d = nc.dram_tensor("xsT", [K, B], in_dt, kind="ExternalInput")
    wnT_d = nc.dram_tensor("wnT", [K, CSH], in_dt, kind="ExternalInput")
    out_d = nc.dram_tensor("out", [2, 128, 8 * ncol], f32, kind="ExternalOutput")

    with tile.TileContext(nc) as tc:
        with (
            tc.tile_pool(name="xs", bufs=1) as xs_pool,
            tc.tile_pool(name="w", bufs=wbufs or W_BUFS) as w_pool,
            tc.tile_pool(name="wt", bufs=1) as wt_pool,
            tc.tile_pool(name="ps", bufs=4096 // cg, space=bass.MemorySpace.PSUM) as ps_pool,
            tc.tile_pool(name="ex", bufs=exbufs or EX_BUFS) as ex_pool,
            tc.tile_pool(name="m1", bufs=2) as m1_pool,
            tc.tile_pool(name="m2", bufs=2) as m2_pool,
            tc.tile_pool(name="tx", bufs=2) as tx_pool,
            tc.tile_pool(name="st", bufs=1) as st_pool,
        ):
            # xs resident in SBUF as [p, kc, b]: k = kc*128 + p; two half-batch
            # DMAs so the first matmuls start after half the transfer.
            xs_sb = xs_pool.tile([128, 4, B], in_dt, tag="xs")
            xsT_r = xsT_d.ap().rearrange("(kc p) b -> p kc b", p=128)
            for half in range(2):
                sl = slice(half * (B // 2), (half + 1) * (B // 2))
                nc.sync.dma_start(xs_sb[:, :, sl], xsT_r[:, :, sl])

            # source view of wnT with partition inside: [p, kc, c]
            wnT_r = wnT_d.ap().rearrange("(kc p) c -> p kc c", p=128)

            # fast-exp region weights loaded once
            wt_sb = wt_pool.tile([128, 4, fx], in_dt, tag="wt")
            nc.sync.dma_start(wt_sb[:], wnT_r[:, :, fx0:CSH])

            # per-(b_chunk, group) partial stats; column b*ncol + gi
            sumbuf = st_pool.tile([128, 8 * ncol], f32, tag="sumbuf")
            maxbuf = st_pool.tile([128, 8 * ncol], f32, tag="maxbuf")
            nc.gpsimd.memset(maxbuf[:], 0.0)
            if skip or sum_mode == "none":
                nc.gpsimd.memset(sumbuf[:], 0.0)


            def emit_fx(b):
                # fast-exp region for b-chunk b: dve_cols + TAIL classes via a
                # DVE Schraudolph fast-exp (ScalarE is the bottleneck engine;
                # this rebalances work onto VectorE's freed capacity)
                ps = ps_pool.tile([128, cg], f32, tag="ps")
                for g in range(2):
                    for h0 in range(0, fx, mm_n):
                        hsz = min(mm_n, fx - h0)
                        nc.tensor.matmul(
                            ps[:, h0 : h0 + hsz],
                            xs_sb[:, 2 * g : 2 * g + 2, b * 128 : b * 128 + 128],
                            wt_sb[:, 2 * g : 2 * g + 2, h0 : h0 + hsz],
                            start=(g == 0),
                            stop=(g == 1),
                            perf_mode=mybir.MatmulPerfMode.DoubleRow,
                            skip_group_check=True,
                        )
                scol = sumbuf[:, b * ncol + ng : b * ncol + ng + 1]
                mcol = maxbuf[:, b * ncol + ng : b * ncol + ng + 1]
                if "dve" in skip:
                    ex = ex_pool.tile([128, cg], bf16, tag="ex")
                    nc.scalar.activation(
                        ex[:, :8], ps[:, :8],
                        mybir.ActivationFunctionType.Copy, scale=1.0,
                    )
                else:
                    tx = tx_pool.tile([128, fx], i16, tag="tx")
                    nc.vector.tensor_scalar(
                        tx[:], ps[:, :fx], FE_A, FE_B,
                        mybir.AluOpType.mult, mybir.AluOpType.add,
                    )
                    nc.vector.tensor_reduce(
                        scol, tx[:].bitcast(bf16), axis=mybir.AxisListType.X,
                        op=mybir.AluOpType.add,
                    )
                    nc.vector.tensor_reduce(
                        mcol, tx[:], axis=mybir.AxisListType.X,
                        op=mybir.AluOpType.max,
                    )

            for _rep in range(repeat):
                for gi in range(ng):
                    c0 = gi * cg
                    w = widths[gi]
                    w_t = w_pool.tile([128, 4, cg], in_dt, tag="w")
                    # chunked so the first matmuls (sub-tile range deps) start
                    # as soon as their slice lands, shortening the ramp
                    nw = 4 if gi == 0 else 2
                    cw = (w + nw - 1) // nw
                    for ci in range(nw):
                        lo, hi = ci * cw, min((ci + 1) * cw, w)
                        if lo >= hi:
                            continue
                        nc.sync.dma_start(
                            w_t[:, :, lo:hi], wnT_r[:, :, c0 + lo : c0 + hi]
                        )
                    for b in range(8):
                        ps = ps_pool.tile([128, cg], f32, tag="ps")
                        # g outer / h inner: consecutive matmuls share the
                        # stationary operand (same b, g), easing LDWEIGHTS
                        # pressure; PSUM has_written bits handle the
                        # interleaved accumulation groups per h-slice.
                        for g in range(2):
                            for h0 in range(0, w, mm_n):
                                hsz = min(mm_n, w - h0)
                                nc.tensor.matmul(
                                    ps[:, h0 : h0 + hsz],
                                    xs_sb[:, 2 * g : 2 * g + 2, b * 128 : b * 128 + 128],
                                    w_t[:, 2 * g : 2 * g + 2, h0 : h0 + hsz],
                                    start=(g == 0),
                                    stop=(g == 1),
                                    perf_mode=mybir.MatmulPerfMode.DoubleRow,
                                    skip_group_check=True,
                                )
                        scol = sumbuf[:, b * ncol + gi : b * ncol + gi + 1]
                        if "act" in skip:
                            ex = ex_pool.tile([128, cg], bf16, tag="ex")
                            nc.scalar.activation(
                                ex[:, :8], ps[:, :8],
                                mybir.ActivationFunctionType.Copy, scale=1.0,
                            )
                        else:
                            ex = ex_pool.tile([128, cg], bf16, tag="ex")
                            nc.scalar.activation(
                                ex[:, :w],
                                ps[:, :w],
                                mybir.ActivationFunctionType.Exp,
                                scale=S,
                                accum_out=scol if sum_mode == "accum" else None,
                            )
                        mcol = maxbuf[:, b * ncol + gi : b * ncol + gi + 1]
                        if "dve" not in skip and "act" not in skip:
                            # free-axis max of the bf16 exp tile (exp monotone
                            # in cosine): shrink 8x with three 2x-packed
                            # tensor_tensor maxes, then a 1x-rate reduce.
                            # (DMA-ing partial-max slabs to the host instead
                            # costs ~3MB extra DMA and measured +15us.)
                            h1, h2, h3 = w // 2, w // 4, w // 8
                            t1 = m1_pool.tile([128, cg // 2], bf16, tag="mx1")
                            nc.vector.tensor_max(t1[:, :h1], ex[:, :h1], ex[:, h1:w])
                            t2 = m2_pool.tile([128, cg // 4], bf16, tag="mx2")
                            nc.vector.tensor_max(t2[:, :h2], t1[:, :h2], t1[:, h2:h1])
                            t3 = m2_pool.tile([128, cg // 8], bf16, tag="mx3")
                            nc.vector.tensor_max(t3[:, :h3], t2[:, :h3], t2[:, h3:h2])
                            nc.vector.tensor_reduce(
                                mcol, t3[:, :h3], axis=mybir.AxisListType.X,
                                op=mybir.AluOpType.max,
                            )

                        # interleave one fast-exp unit per three group slots
                        # so its VectorE work hides under ScalarE's exp groups
                        # instead of forming a ScalarE-idle tail phase
                        slot = gi * 8 + b
                        if slot % 3 == 2:
                            emit_fx(slot // 3)

            out_ap = out_d.ap()
            nc.sync.dma_start(out_ap[0], sumbuf[:])
            nc.sync.dma_start(out_ap[1], maxbuf[:])

    nc.compile()
    return nc


def get_nc(repeat=1, skip=(), cg=None, mm_n=None, sum_mode="accum", exbufs=None, wbufs=None):
    key = (repeat, tuple(skip), cg or CG, mm_n or 512, sum_mode,
           exbufs or EX_BUFS, wbufs or W_BUFS)
    if key not in _NC_CACHE:
        _NC_CACHE[key] = build_nc(repeat, skip, cg, mm_n, sum_mode, exbufs, wbufs)
    return _NC_CACHE[key]


def quantize_host(x, w):
    """Host prep: fold squashing scale into x, L2 norm into w; quantize fp8;
    lay out as [K, B] / [K, C] with K rows (k = kc*128 + p after rearrange)."""
    qdt = ml_dtypes.float8_e4m3
    sq = np.einsum("bk,bk->b", x, x)
    xs = x * (np.sqrt(sq) / (sq + 1.0))[:, None]
    wn = w / np.sqrt(np.einsum("ck,ck->c", w, w))[:, None]
    xs_q = xs.astype(qdt)
    wn_q = wn.astype(qdt)
    xsT = np.ascontiguousarray(xs_q.T)  # [K, B]
    wnT = np.ascontiguousarray(wn_q.T)  # [K, C]
    return xs_q, wn_q, xsT, wnT


def make_in_maps(input, label, weight):
    x = np.asarray(input, dtype=np.float64)
    w = np.asarray(weight, dtype=np.float64)
    _, _, xsT, wnT = quantize_host(x, w)
    return [
        {"xsT": xsT, "wnT": np.ascontiguousarray(wnT[:, i * CSH : (i + 1) * CSH])}
        for i in range(NCORES)
    ]


def kernel(input, label, weight):
    x = np.asarray(input, dtype=np.float64)  # [B, K]
    lab = np.asarray(label).astype(np.int64)  # [B]
    w = np.asarray(weight, dtype=np.float64)  # [C, K]

    xs_q, wn_q, xsT, wnT = quantize_host(x, w)

    in_maps = [
        {"xsT": xsT, "wnT": np.ascontiguousarray(wnT[:, i * CSH : (i + 1) * CSH])}
        for i in range(NCORES)
    ]

    nc = get_nc()
    results = run_bass_kernel_spmd(nc, in_maps, core_ids=list(range(NCORES))).results

    # combine per-core partials: out[s][p, b*NCOL + g] is the stat for batch row
    # b*128 + p, class-group g (g<NG: ScalarE true-exp domain; g==NG: the
    # fast-exp region in Schraudolph bf16-pattern / fast-exp value domain)
    SE = np.zeros(B, dtype=np.float64)
    MXE = np.full(B, 0.0)    # max of exp(S*cos) over ACT groups (bf16-rounded)
    MXP = np.full(B, 0.0)    # max of fast-exp bit pattern over the DVE region
    for r in results:
        o = np.asarray(r["out"], dtype=np.float64)  # [2, 128, 8*NCOL]
        sums = o[0].T.reshape(8, NCOL, 128)   # [b_chunk, col, p]
        maxs = o[1].T.reshape(8, NCOL, 128)
        SE += sums.sum(axis=1).reshape(B)
        MXE = np.maximum(MXE, maxs[:, :NG].max(axis=1).reshape(B))
        MXP = np.maximum(MXP, maxs[:, NG].reshape(B))

    # row max cosine from the two max domains
    cos_act = np.log(np.maximum(MXE, 1e-300)) / S
    cos_tail = (MXP - FE_B) / FE_A
    cos_max = np.maximum(cos_act, cos_tail)

    # label-column correction on host, with the same quantized values the device saw
    xs_f = xs_q.astype(np.float64)
    wn_f = wn_q.astype(np.float64)
    coslab = np.einsum("bk,bk->b", xs_f, wn_f[lab])
    sine = np.sqrt(np.clip(1.0 - coslab * coslab, 0.0, 1.0))
    phi = np.where(coslab > TH, coslab * COS_M - sine * SIN_M, coslab - MM)

    total = SE - np.exp(S * coslab) + np.exp(S * phi)
    loss = np.mean(np.log(total) - S * phi)
    # acc: is the label's cosine the row max (slack covers bf16 rounding of the
    # exp domain, fast-exp pattern quantization, and fp22 matmul accumulation)
    acc = 100.0 * np.mean(coslab >= cos_max - 3e-4)

    return (np.float32(loss), np.float32(acc))
